# revision 1
# baseline (speedup 1.0000x reference)
"""Trainium2 Bass kernel for LocalizationLoss (box MSE + cross-entropy, batch mean).

Input : output [262144, 1004] f32  (cols 0:4 = box pred cx,cy,w,h; cols 4:1004 = logits)
        target [262144, 5]    f32  (xmin,ymin,xmax,ymax,class_id)
Output: scalar f32 = mean_b( mean_4((box_pred-box_true)^2) + CE(logits, class) )

Strategy (pure data parallel over 8 cores, 32768 rows each):
  - rows mapped p-major: partition p owns rows p*256..p*256+255 of its shard
  - stream 32 groups of 8 row-tiles [128, 8, 1004]; one big DMA per group
  - ScalarE: exp over logits with fused row-sum (accum_out -> PSUM) = sumexp
  - VectorE: picked logit via one scalar_tensor_tensor per tile:
        out = (iota is_equal class_p) * logits ; accum_out = logits[p, class_p]
    (iota is a [128,1000] constant input 0..999 per row; class_p is the f32
     class id as a per-partition scalar AP)
  - GpSimdE: box-error terms per group as doubled differences (TensorTensor
    only); ScalarE Square(scale=0.5) with accum_out sums all 4 components
  - epilogue: logZ = Ln(sumexp) with fused sum; CE_sum = logZ_sum - picked_sum
  - each core returns [128,1] per-partition partial sums; host adds and /B

This container's walrus build accepts at most ONE sync-wait per instruction,
while the Tile scheduler attaches several. `_split_multiwait_bir` rewrites the
serialized BIR to hoist extra waits onto single-wait NoOp carriers, and is
installed as a wrapper around compile_bir_kernel at import time. The same
walrus also cannot lower the custom-DVE ISA ops (tensor_mask_reduce etc.) or
Pool-engine TensorScalarPtr, so only standard opcodes are used.
"""

import json as _json

import numpy as np

import concourse.bass as bass
import concourse.tile as tile
from concourse import mybir
import concourse.bass_utils as _bass_utils
import concourse.bass2jax as _bass2jax
from concourse.bass_utils import run_bass_kernel_spmd

P = 128
B = 262144
C = 1004
NCLS = 1000
NCORES = 8
R = B // NCORES       # 32768 rows per core
T = R // P            # 256 row-tiles per core (rows per partition)
G = 8                 # row-tiles per group
NG = T // G           # 32 groups

F32 = mybir.dt.float32
ALU = mybir.AluOpType
ACTF = mybir.ActivationFunctionType


# --------------------------------------------------------------------------
# BIR post-pass: this image's walrus supports only one sync-wait per
# instruction; split extras onto NoOp carriers placed just before.
# --------------------------------------------------------------------------
def _split_multiwait_bir(bir_json: bytes) -> bytes:
    d = _json.loads(bir_json)
    changed = False
    for fn in d.get("functions", []):
        for blk in fn.get("blocks", []):
            insts = blk.get("instructions", [])
            out = []
            for ins in insts:
                si = ins.get("sync_info") or {}
                waits = si.get("on_wait") or []
                if len(waits) > 1:
                    changed = True
                    for i, w in enumerate(waits[:-1]):
                        out.append(
                            {
                                "debug": ins.get("debug", 0),
                                "engine": ins["engine"],
                                "ins": [],
                                "name": f"{ins['name']}-wsplit{i}",
                                "opcode": "NoOp",
                                "outs": [],
                                "sync_info": {"on_update": [], "on_wait": [w]},
                            }
                        )
                    ins["sync_info"]["on_wait"] = [waits[-1]]
                out.append(ins)
            blk["instructions"] = out
    if not changed:
        return bir_json
    return _json.dumps(d).encode()


_orig_compile_bir_kernel = _bass_utils.compile_bir_kernel


def _compile_bir_kernel_fixed(bir_json, tmpdir, neff_name="file.neff"):
    if isinstance(bir_json, str):
        bir_json = bir_json.encode()
    return _orig_compile_bir_kernel(_split_multiwait_bir(bir_json), tmpdir, neff_name)


if _bass_utils.compile_bir_kernel is not _compile_bir_kernel_fixed:
    _bass_utils.compile_bir_kernel = _compile_bir_kernel_fixed
    _bass2jax.compile_bir_kernel = _compile_bir_kernel_fixed


# --------------------------------------------------------------------------
# kernel build
# --------------------------------------------------------------------------
def build():
    nc = bass.Bass()
    x = nc.dram_tensor("x", [R, C], F32, kind="ExternalInput")
    t = nc.dram_tensor("t", [R, 5], F32, kind="ExternalInput")
    iota_in = nc.dram_tensor("iota", [P, NCLS], F32, kind="ExternalInput")
    out = nc.dram_tensor("partial", [P, 1], F32, kind="ExternalOutput")

    xv = x[:].rearrange("(p n) c -> p n c", p=P)   # [128, 256, 1004]
    tv = t[:].rearrange("(p n) f -> p n f", p=P)   # [128, 256, 5]

    with tile.TileContext(nc) as tc:
        with (
            tc.tile_pool(name="data", bufs=4) as data_pool,
            tc.tile_pool(name="scr", bufs=2) as scr_pool,
            tc.tile_pool(name="acc", bufs=1) as acc_pool,
        ):
            iota_t = acc_pool.tile([P, NCLS], F32)
            nc.sync.dma_start(out=iota_t, in_=iota_in[:])
            # whole per-core target resident: [128, 256, 5] = 5 KiB/partition,
            # one DMA with contiguous 5120B per-partition chunks
            tgt = acc_pool.tile([P, T, 5], F32)
            nc.sync.dma_start(out=tgt, in_=tv)

            # variable group sizes: small head groups shrink the pipeline
            # fill (compute starts after ~1MB instead of ~4MB), small tail
            # groups shrink the end-of-run compute drain
            group_sizes = [2, 2, 4] + [8] * 30 + [4, 2, 2]
            assert sum(group_sizes) == T
            n_groups = len(group_sizes)
            # tiles whose sumexp goes ACT-exp + DVE-reduce (engine balance)
            dve_sumexp_groups = {0, 6, 12, 18, 24, 30}

            sumexp_all = acc_pool.tile([P, T], F32)      # per-row sum(exp(logits))
            loc_all = acc_pool.tile([P, n_groups], F32)  # per-group sq-err sums
            picked_all = acc_pool.tile([P, T], F32)      # per-row logits[class]

            t0 = 0
            for grp, gs in enumerate(group_sizes):
                data = data_pool.tile([P, gs, C], F32, tag="data")
                nc.sync.dma_start(out=data, in_=xv[:, t0 : t0 + gs, :])

                # box-error terms as doubled differences (GpSimd TensorTensor
                # on [128, G, 2] views), then squared on GpSimd and summed by
                # one small VectorE reduce:
                #   e_cx_cy = (t01 + t23) - 2*bp01      -> (0.5*e)^2 = err^2
                #   e_wh    = 2*((t23 - t01) - bp23)    -> (0.5*e)^2 = err^2
                e4 = scr_pool.tile([P, 2, gs, 2], F32, tag="e4")
                u2 = scr_pool.tile([P, gs, 2], F32, tag="u2")
                t01 = tgt[:, t0 : t0 + gs, 0:2]
                t23 = tgt[:, t0 : t0 + gs, 2:4]
                bp01 = data[:, :, 0:2]
                bp23 = data[:, :, 2:4]
                nc.gpsimd.tensor_add(u2, t01, t23)
                nc.gpsimd.tensor_sub(u2, u2, bp01)
                nc.gpsimd.tensor_sub(e4[:, 0, :, :], u2, bp01)
                nc.gpsimd.tensor_sub(u2, t23, t01)
                nc.gpsimd.tensor_sub(u2, u2, bp23)
                nc.gpsimd.tensor_add(e4[:, 1, :, :], u2, u2)
                nc.gpsimd.tensor_mul(e4, e4, e4)
                nc.vector.tensor_reduce(
                    out=loc_all[:, grp : grp + 1], in_=e4,
                    axis=mybir.AxisListType.XYZ, op=ALU.add,
                )

                for g in range(gs):
                    tt = t0 + g
                    exp_scr = scr_pool.tile([P, NCLS], F32, tag="exp_scr")
                    # ScalarE is the busiest engine; for a slice of tiles do
                    # exp without the accumulator (saves the ~280ns
                    # READ_ACCUMULATOR per tile) and let VectorE reduce.
                    if grp in dve_sumexp_groups and g == 0:
                        nc.scalar.activation(
                            out=exp_scr, in_=data[:, g, 4:C], func=ACTF.Exp
                        )
                        nc.vector.tensor_reduce(
                            out=sumexp_all[:, tt : tt + 1],
                            in_=exp_scr,
                            axis=mybir.AxisListType.X,
                            op=ALU.add,
                        )
                    else:
                        nc.scalar.activation(
                            out=exp_scr,
                            in_=data[:, g, 4:C],
                            func=ACTF.Exp,
                            accum_out=sumexp_all[:, tt : tt + 1],
                        )
                    pick_scr = scr_pool.tile([P, NCLS], F32, tag="pick_scr")
                    nc.vector.scalar_tensor_tensor(
                        pick_scr,
                        iota_t,
                        tgt[:, tt, 4:5],
                        data[:, g, 4:C],
                        ALU.is_equal,
                        ALU.mult,
                        accum_out=picked_all[:, tt : tt + 1],
                    )
                t0 += gs

            # ---- epilogue ----
            logz_scr = acc_pool.tile([P, T], F32)
            logz_sum = acc_pool.tile([P, 1], F32)
            nc.scalar.activation(
                out=logz_scr, in_=sumexp_all, func=ACTF.Ln, accum_out=logz_sum
            )
            pick_sum = acc_pool.tile([P, 1], F32)
            nc.vector.tensor_reduce(
                out=pick_sum, in_=picked_all, axis=mybir.AxisListType.X, op=ALU.add
            )
            loc_sum = acc_pool.tile([P, 1], F32)
            nc.vector.tensor_reduce(
                out=loc_sum, in_=loc_all, axis=mybir.AxisListType.X, op=ALU.add
            )
            s = acc_pool.tile([P, 1], F32)
            # loc_all holds (2*err)^2 sums -> mean over 4 comps with the
            # doubling correction is 0.25 * 0.25 = 0.0625
            nc.vector.scalar_tensor_tensor(
                s, loc_sum, 0.0625, logz_sum, ALU.mult, ALU.add
            )
            nc.vector.tensor_sub(s, s, pick_sum)
            nc.sync.dma_start(out=out[:], in_=s)

    return nc


_IOTA = np.ascontiguousarray(
    np.broadcast_to(np.arange(NCLS, dtype=np.float32), (P, NCLS))
)


def _run(output, target, **spmd_kwargs):
    output = np.ascontiguousarray(np.asarray(output, dtype=np.float32))
    target = np.ascontiguousarray(np.asarray(target, dtype=np.float32))
    assert output.shape == (B, C), output.shape
    assert target.shape == (B, 5), target.shape
    nc = build()
    in_maps = [
        {
            "x": output[i * R : (i + 1) * R],
            "t": target[i * R : (i + 1) * R],
            "iota": _IOTA,
        }
        for i in range(NCORES)
    ]
    res = run_bass_kernel_spmd(nc, in_maps, core_ids=list(range(NCORES)), **spmd_kwargs)
    total = 0.0
    for r in res.results:
        total += r["partial"].astype(np.float64).sum()
    return np.float32(total / B), res


def kernel(output, target):
    val, _ = _run(output, target)
    return np.asarray(val, dtype=np.float32)


def kernel_profiled(output, target, **kw):
    """Returns (scalar, BassKernelResults) with trace for perf analysis."""
    return _run(output, target, trace=True, **kw)

